# revision 21
# baseline (speedup 1.0000x reference)
"""Trainium2 Bass kernel for a single causal-attention transformer block.

Reference computation (per batch element b):
    xn  = rms_norm(x[b]) * rms_w
    q/k/v = xn @ Wq/Wk/Wv            (16 heads x 128 head dim)
    att = causal_softmax(q k^T / sqrt(2048)) @ v
    out[b] = att @ Wo + x[b]

Sharding (8 NeuronCores): tensor-parallel over heads x data-parallel over
batch.  Core c handles batch b = c // 4 and head-group i = c % 4 (4 heads,
512 columns of Wq/Wk/Wv, 512 rows of Wo).  Each core computes a partial
output  att_i @ Wo_i  for its batch element; the host sums the 4 partials
per batch (scaled by 2^-10, see below) and adds the residual.

fp8 DoubleRow scheme: all heavy matmuls except the scores run as
float8e4 (e4m3) MatmulPerfMode.DoubleRow, which contracts 256 elements
per instruction (two 128-deep planes packed in an extra free dim of 2)
at ~0.43 ns/output-column -- 2x the bf16 rate, measured 154 TF/s.

  - x arrives pre-transposed/quantized from the host as pair tiles
    xp[128, j, i, s] = x^T[(2j+i)*128 + p, s] (fp8).
  - Wq/Wk/Wv/Wo arrive fp8, scaled by 2^5 to avoid e4m3 subnormals
    (|W| ~ 0.022 would quantize terribly at 2^-10 granularity).
  - RMS stats: squares split across DVE/ACT/gpsimd (fp8), column sums
    via an all-ones (value 2^-5) DoubleRow matmul; sqrt/reciprocal
    produce rstd_b = rstd * 2^-5 broadcast on all partitions, so the
    q/k PSUM evacuation (x 2^5 Wq^T x) * rstd_b lands TRUE q/k in bf16.
  - scores stay bf16 (contraction is only dh=128, DoubleRow can't pair);
    the 1/sqrt(2048) score scale is applied inside the exp activation,
    which writes fp8 probs directly.
  - causal mask: affine_select on fp8 probs for diagonal tiles; the
    partner plane's never-written [c0p:c0) strip is memset to 0 before
    the paired PV matmul streams it.
  - PV + softmax-denominator accumulate per query-chunk via DoubleRow
    over t-tile pairs (v in fp8 pair tiles, ones value 2^-5), so the
    full-lane reciprocal yields 2^5/den and the attn evacuation writes
    2^5*attn in fp8 (healthy range; raw attn ~ 1/sqrt(L) is subnormal).
  - o_proj: DoubleRow over the 4 dh-blocks (2 pairs); PSUM holds
    2^10 * partial, evacuated as bf16 and divided by 2^10 on the host.

PSUM chain rule (hardware): a PSUM bank supports only ONE open
accumulation group at a time -- all start/stop chains sharing a
[128,512] tile run to completion before the tile's other half starts.

Scheduling: the PE queue executes in program order, so the exp (ACT)
latency is hidden by emitting score matmuls up to two query-chunks
ahead of the PV/denominator chains that consume them:
  ss | q | k | S(0) | S(1) | transposes | v | S(2) | PV0 o0 | S(3) |
  PV1 o1 | PV2 o2 | PV3 o3
(ACT's total exp work, ~110us, is just under the PE's ~130us in that
window; without the lookahead the PE stalls on exp at every head.)

Measured HW time ~230 us across 8 cores (slowest core) vs 377 us for
the all-bf16 version; end-to-end absmax error ~1.2e-2 of the output
absmax (fp8 quantization noise; threshold is 2e-2).
"""

import numpy as np
import ml_dtypes

S = 2048          # sequence length
HID = 2048        # hidden dim
NH = 16           # total heads
DH = 128          # head dim
TP = 4            # head-group shards
DP = 2            # batch shards
KSH = HID // TP   # per-core key-dim shard (512)
NHS = KSH // DH   # heads per core (4)
NT = S // 128     # 128-row tiles along s/t (16)
NSC = S // 512    # 512-wide chunks along s (4)
NJP = HID // 256  # hidden-dim pairs (8)
EPS = 1e-5
WS = 32.0         # fp8 weight pre-scale (2^5)
OSC = 1.0 / (WS * WS)  # host-side unscale of output partials

_STATE = {}


def _build_nc():
    from contextlib import ExitStack

    import concourse.bacc as bacc
    import concourse.tile as tile
    from concourse import mybir

    F32 = mybir.dt.float32
    BF = mybir.dt.bfloat16
    F8 = mybir.dt.float8e4
    AF = mybir.ActivationFunctionType
    PM = mybir.MatmulPerfMode
    DR = PM.DoubleRow
    SCALE = 1.0 / float(np.sqrt(np.float32(HID)))

    nc = bacc.Bacc("TRN2")
    xp = nc.dram_tensor("xp", [128, NJP, 2, S], F8, kind="ExternalInput")
    wq = nc.dram_tensor("wq", [128, NJP, 2, KSH], F8, kind="ExternalInput")
    wk = nc.dram_tensor("wk", [128, NJP, 2, KSH], F8, kind="ExternalInput")
    wv = nc.dram_tensor("wv", [128, NJP, 2, KSH], F8, kind="ExternalInput")
    wo = nc.dram_tensor("wo", [128, 2, 2, HID], F8, kind="ExternalInput")
    out = nc.dram_tensor("out", [S, HID], BF, kind="ExternalOutput")

    with tile.TileContext(nc) as tc, ExitStack() as ctx:
        misc = ctx.enter_context(tc.tile_pool(name="misc", bufs=1))
        qt_pool = ctx.enter_context(tc.tile_pool(name="qt", bufs=NHS))
        kt_pool = ctx.enter_context(tc.tile_pool(name="kt", bufs=NHS))
        v_pool = ctx.enter_context(tc.tile_pool(name="v", bufs=NJP))
        attn_pool = ctx.enter_context(tc.tile_pool(name="attn", bufs=2))
        # early pool: probs for sc=0/1/2 (coexists with the phase-A/B pools)
        probs_early = ctx.enter_context(tc.tile_pool(name="probsE", bufs=50))
        denb_pool = ctx.enter_context(tc.tile_pool(name="denb", bufs=3))
        wo_pool = ctx.enter_context(tc.tile_pool(name="wo", bufs=1, side="right"))
        out_pool = ctx.enter_context(tc.tile_pool(name="outp", bufs=3, side="right"))

        # all-ones (value 2^-5) DoubleRow stationary: used for the RMS
        # column sums and the softmax denominators
        ones8 = misc.tile([128, 2, 128], F8, tag="ones8", name="ones8")
        nc.vector.memset(ones8, 1.0 / WS)
        eps_sb = misc.tile([128, 1], F32, tag="eps_sb", name="eps_sb")
        nc.vector.memset(eps_sb, EPS * WS * WS)
        rstd_b = misc.tile([128, S], F32, tag="rstd_b", name="rstd_b")
        # rstd transposed to per-partition layout: rstd_colT[p, st] = rstd[st*128+p]
        rstd_colT = misc.tile([128, NT], F32, tag="rstd_colT", name="rstd_colT")
        ident = misc.tile([128, 128], F32, tag="ident", name="ident")
        nc.vector.memset(ident, 1.0)
        nc.gpsimd.affine_select(
            out=ident, in_=ident, compare_op=mybir.AluOpType.is_equal,
            fill=0.0, base=0, channel_multiplier=1, pattern=[[-1, 128]],
        )

        pp = ctx.enter_context(tc.tile_pool(name="pp", bufs=5, space="PSUM"))
        # dedicated pool for score tiles: their buffers recycle at exp (ACT)
        # rate, and a shared FIFO would head-of-line block the PV chains
        pp_s = ctx.enter_context(tc.tile_pool(name="pps", bufs=3, space="PSUM"))

        qts, kts, vp = [], [], []
        attnp = [attn_pool.tile([128, 2, S], F8, tag="attn", name="attn")
                 for _ in range(2)]

        # ---- attention emitters (closures; called in the schedule below) ----
        def emit_scores_head(sc, hd, pool):
            """Scores + exp + mask for one head of chunk sc -> probs tiles."""
            npair = 2 * (sc + 1)
            if True:
                tiles_h = []
                for jp in range(npair):
                    # pair truncation: columns below c0p are fully masked
                    # in BOTH planes of this t-tile pair
                    c0p = max(0, 128 * (2 * jp - 4 * sc))
                    ptp = pool.tile([128, 2, 512], F8, tag="probs", name="probs")
                    tiles_h.append(ptp)
                    for ii in range(2):
                        tt = 2 * jp + ii
                        jd = tt - 4 * sc
                        c0 = 128 * jd if jd > 0 else 0
                        ps_s = pp_s.tile([128, 512], F32, tag="pps", name="ps")
                        nc.tensor.matmul(
                            ps_s[:, c0:],
                            kts[hd][:, tt * 128:(tt + 1) * 128],
                            qts[hd][:, sc * 512 + c0:(sc + 1) * 512],
                            start=True,
                            stop=True,
                        )
                        nc.scalar.activation(
                            ptp[:, ii, c0:], ps_s[:, c0:], AF.Exp, scale=SCALE
                        )
                        if jd >= 0:
                            # keep where col >= t + 128*jd
                            nc.gpsimd.affine_select(
                                out=ptp[:, ii, c0:],
                                in_=ptp[:, ii, c0:],
                                compare_op=mybir.AluOpType.is_ge,
                                fill=0.0,
                                base=c0 - 128 * jd,
                                channel_multiplier=-1,
                                pattern=[[1, 512 - c0]],
                            )
                        if c0p < c0:
                            # partner plane of the pair streams [c0p:c0);
                            # zero this never-written strip
                            nc.gpsimd.memset(ptp[:, ii, c0p:c0], 0.0)
            return tiles_h

        def emit_pv_head(sc, hd, probs_h):
            """PV + denominator chains + attn evacuation for one head."""
            npair = 2 * (sc + 1)
            cs = slice(sc * 512, (sc + 1) * 512)
            if True:
                ps_at = pp.tile([128, 512], F32, tag="pp", name="at")
                ps_dn = pp.tile([128, 512], F32, tag="pp", name="dn")
                # one open accumulation chain per PSUM bank at a time
                for qc in range(2):
                    hi = (qc + 1) * 256
                    jlist = [jp for jp in range(npair)
                             if max(0, 128 * (2 * jp - 4 * sc)) < hi]
                    for idx, jp in enumerate(jlist):
                        c0p = max(0, 128 * (2 * jp - 4 * sc))
                        lo = max(qc * 256, c0p)
                        st_, sp_ = (idx == 0), (idx == len(jlist) - 1)
                        nc.tensor.matmul(
                            ps_at[:, lo:hi],
                            vp[jp][:, :, hd * 128:(hd + 1) * 128],
                            probs_h[jp][:, :, lo:hi],
                            start=st_, stop=sp_, perf_mode=DR,
                        )
                        nc.tensor.matmul(
                            ps_dn[:, lo:hi],
                            ones8,
                            probs_h[jp][:, :, lo:hi],
                            start=st_, stop=sp_, perf_mode=DR,
                        )
                denb = denb_pool.tile([128, 512], F32, tag="denb", name="denb")
                nc.vector.reciprocal_approx_fast(denb, ps_dn)
                # 2^5 * attn (ps_dn carries 2^-5); healthy fp8 range
                nc.vector.tensor_mul(attnp[hd // 2][:, hd % 2, cs], ps_at, denb)

        def emit_oproj(sc, wot, split_evac=False):
            for st in range(4 * sc, 4 * sc + 4):
                ps_o = [pp.tile([128, 512], F32, tag="pp", name="po")
                        for _ in range(4)]
                for h in range(2):
                    for c in range(4):
                        ch = 2 * c + h
                        for jp in range(2):
                            nc.tensor.matmul(
                                ps_o[c][:, h * 256:h * 256 + 256],
                                attnp[jp][:, :, st * 128:(st + 1) * 128],
                                wot[:, jp, :, ch * 256:(ch + 1) * 256],
                                start=(jp == 0),
                                stop=(jp == 1),
                                perf_mode=DR,
                            )
                ot = out_pool.tile([128, HID], BF, tag="outp", name="outp")
                for ec in range(4):
                    es = slice(ec * 512, (ec + 1) * 512)
                    if split_evac and ec % 2 == 1:
                        # ACT is exp-idle by the late chunks; halving the
                        # serial DVE evac latency shortens the kernel tail
                        nc.scalar.activation(ot[:, es], ps_o[ec], AF.Copy)
                    else:
                        nc.vector.tensor_copy(ot[:, es], ps_o[ec])
                    nc.sync.dma_start(
                        out=out[st * 128:(st + 1) * 128, es], in_=ot[:, es]
                    )

        # ---------------- phases A+B (RMS stats + projections) ----------------
        with ExitStack() as ab:
            xp_pool = ab.enter_context(tc.tile_pool(name="xp", bufs=NJP, side="right"))
            w_pool = ab.enter_context(tc.tile_pool(name="wstream", bufs=3, side="right"))

            xpt = []
            for j in range(NJP):
                t = xp_pool.tile([128, 2, S], F8, tag="xp", name="xp")
                # split halves -> parallel DMA queues
                nc.sync.dma_start(out=t[:, 0, :], in_=xp[:, j, 0, :])
                nc.sync.dma_start(out=t[:, 1, :], in_=xp[:, j, 1, :])
                xpt.append(t)
            w8 = {}
            for name, dram in (("wq", wq), ("wk", wk), ("wv", wv)):
                wt = w_pool.tile([128, NJP, 2, KSH], F8, tag="w", name=name)
                nc.sync.dma_start(out=wt, in_=dram[:, :, :, :])
                w8[name] = wt
            wot = wo_pool.tile([128, 2, 2, HID], F8, tag="wo", name="wo")
            nc.sync.dma_start(out=wot, in_=wo[:, :, :, :])

            # squares -> DoubleRow ones-matmul column sums; sq tiles live
            # only inside this nested scope so their SBUF frees early.
            # Done in two s-halves so the sq pool is 16KB instead of 32KB.
            with ExitStack() as sqs_scope:
                sq_pool = sqs_scope.enter_context(
                    tc.tile_pool(name="sq", bufs=NJP, side="right")
                )
                ss = [pp.tile([128, 512], F32, tag="pp", name="ss")
                      for _ in range(4)]
                for sh in range(2):
                    sqs = []
                    for j in range(NJP):
                        sq = sq_pool.tile([128, 2, S // 2], F8, tag="sq",
                                          name="sq")
                        xsl = xpt[j][:, :, sh * 1024:(sh + 1) * 1024]
                        # split across DVE/ACT/gpsimd so one engine's serial
                        # latency doesn't gate rstd
                        eng = j % 3
                        if eng == 0:
                            nc.vector.tensor_mul(sq, xsl, xsl)
                        elif eng == 1:
                            nc.scalar.activation(sq, xsl, AF.Square)
                        else:
                            nc.gpsimd.tensor_mul(sq, xsl, xsl)
                        sqs.append(sq)
                    for h in range(2):
                        for j in range(NJP):
                            for cl in range(2):
                                c = 2 * sh + cl
                                ch = 2 * c + h
                                off = ch * 256 - 1024 * sh
                                nc.tensor.matmul(
                                    ss[c][:, h * 256:h * 256 + 256],
                                    ones8,
                                    sqs[j][:, :, off:off + 256],
                                    start=(j == 0),
                                    stop=(j == NJP - 1),
                                    perf_mode=DR,
                                )
                for c in range(4):
                    cs = slice(c * 512, (c + 1) * 512)
                    # mtmp = 2^5 * sqrt(ms + eps); reciprocal -> rstd * 2^-5
                    mtmp = denb_pool.tile([128, 512], F32, tag="denb", name="mtmp")
                    nc.scalar.activation(
                        mtmp, ss[c], AF.Sqrt, bias=eps_sb, scale=WS * WS * WS / HID
                    )
                    nc.vector.reciprocal_approx_fast(rstd_b[:, cs], mtmp)

            # --- q/k projections, head by head, each head's sc=0/1
            # scores emitted immediately so ACT's exp stream starts ~40us
            # earlier than an all-projections-first order would allow
            probs0, probs1 = [], []
            for dt in range(NHS):
                for wname, pool, dst_list in (("wq", qt_pool, qts),
                                              ("wk", kt_pool, kts)):
                    w8t = w8[wname]
                    ps4 = [pp.tile([128, 512], F32, tag="pp", name="pq")
                           for _ in range(4)]
                    # j-outer: 4 chains (separate banks) advance together,
                    # so during the x DMA stream the PE has 4 ready matmuls
                    # per arrived tile instead of blocking on xp[7]; the 4
                    # consecutive instrs also share one stationary
                    for h in range(2):
                        for j in range(NJP):
                            for c in range(4):
                                ch = 2 * c + h
                                nc.tensor.matmul(
                                    ps4[c][:, h * 256:h * 256 + 256],
                                    w8t[:, j, :, dt * 128:(dt + 1) * 128],
                                    xpt[j][:, :, ch * 256:(ch + 1) * 256],
                                    start=(j == 0),
                                    stop=(j == NJP - 1),
                                    perf_mode=DR,
                                )
                    dst = pool.tile([128, S], BF, tag="qt", name=wname + "t")
                    for c in range(4):
                        cs = slice(c * 512, (c + 1) * 512)
                        nc.vector.tensor_mul(dst[:, cs], ps4[c], rstd_b[:, cs])
                    dst_list.append(dst)
                probs0.append(emit_scores_head(0, dt, probs_early))
                probs1.append(emit_scores_head(1, dt, probs_early))

            # PE-transpose rstd_b slices to get per-partition rstd columns
            for st in range(NT):
                ptr = pp.tile([128, 512], F32, tag="pp", name="ptr")
                nc.tensor.transpose(
                    ptr[:, 0:128], rstd_b[:, st * 128:(st + 1) * 128], ident
                )
                nc.vector.tensor_copy(rstd_colT[:, st:st + 1], ptr[:, 0:1])

            # --- v projection (interleaved with sc=2 scores): pair
            # tiles vp[tp][p, i, m] = v[(2tp+i)*128+p, m]
            w8v = w8["wv"]
            for _ in range(NJP):
                vp.append(v_pool.tile([128, 2, KSH], F8, tag="v", name="v"))
            probs2 = []
            for g in range(NHS):
                for st in range(4 * g, 4 * g + 4):
                    psv = pp.tile([128, 512], F32, tag="pp", name="pv")
                    for h in range(2):
                        for j in range(NJP):
                            nc.tensor.matmul(
                                psv[:, h * 256:(h + 1) * 256],
                                xpt[j][:, :, st * 128:(st + 1) * 128],
                                w8v[:, j, :, h * 256:(h + 1) * 256],
                                start=(j == 0),
                                stop=(j == NJP - 1),
                                perf_mode=DR,
                            )
                    nc.vector.tensor_scalar_mul(
                        vp[st // 2][:, st % 2, :], psv, rstd_colT[:, st:st + 1]
                    )
                probs2.append(emit_scores_head(2, g, probs_early))
        # xp/wstream released here

        # late pool: probs for sc=3 (fits once the A/B pools are gone)
        probs_late = ctx.enter_context(tc.tile_pool(name="probsL", bufs=34))

        # -------- phases C+D: pipelined attention + o_proj ----------
        # last scores chunk interleaves per-head with PV0 so the PE never
        # sits in a solo score burst waiting on ACT exp
        probs3 = []
        for hd in range(NHS):
            probs3.append(emit_scores_head(3, hd, probs_late))
            emit_pv_head(0, hd, probs0[hd])
        emit_oproj(0, wot)
        for hd in range(NHS):
            emit_pv_head(1, hd, probs1[hd])
        emit_oproj(1, wot)
        for hd in range(NHS):
            emit_pv_head(2, hd, probs2[hd])
        emit_oproj(2, wot, split_evac=True)
        for hd in range(NHS):
            emit_pv_head(3, hd, probs3[hd])
        emit_oproj(3, wot, split_evac=True)

    return nc


def get_nc():
    if "nc" not in _STATE:
        nc = _build_nc()
        nc.finalize()
        _STATE["nc"] = nc
    return _STATE["nc"]


def _pair_pack(a):
    """[256*n, m] -> [128, n, 2, m] with [p, j, i, m] = a[(2j+i)*128 + p, m]."""
    n = a.shape[0] // 256
    return np.ascontiguousarray(
        a.reshape(n, 2, 128, a.shape[1]).transpose(2, 0, 1, 3)
    )


def make_in_maps(x, rms_w, Wq, Wk, Wv, Wo):
    """Host-side sharding: returns one input dict per core (8 cores)."""
    e4 = ml_dtypes.float8_e4m3
    rw = rms_w.astype(np.float32)[:, None]
    wq_f = rw * Wq.astype(np.float32) * WS
    wk_f = rw * Wk.astype(np.float32) * WS
    wv_f = rw * Wv.astype(np.float32) * WS
    wo_f = Wo.astype(np.float32) * WS
    xp_b = [
        _pair_pack(np.ascontiguousarray(x[b].astype(np.float32).T)).astype(e4)
        for b in range(DP)
    ]
    in_maps = []
    for c in range(DP * TP):
        b, i = divmod(c, TP)
        cols = slice(i * KSH, (i + 1) * KSH)
        in_maps.append({
            "xp": xp_b[b],
            "wq": _pair_pack(wq_f[:, cols]).astype(e4),
            "wk": _pair_pack(wk_f[:, cols]).astype(e4),
            "wv": _pair_pack(wv_f[:, cols]).astype(e4),
            "wo": _pair_pack(wo_f[cols, :]).astype(e4),
        })
    return in_maps


def kernel(x, rms_w, Wq, Wk, Wv, Wo, _trace=False, _results_out=None):
    from concourse.bass_utils import run_bass_kernel_spmd

    nc = get_nc()
    in_maps = make_in_maps(x, rms_w, Wq, Wk, Wv, Wo)
    kw = {}
    if _trace:
        kw = dict(trace=True, trace_cores=list(range(DP * TP)))
    res = run_bass_kernel_spmd(
        nc, in_maps, core_ids=list(range(DP * TP)), **kw
    )
    if _results_out is not None:
        _results_out.append(res)
    out = np.empty((DP, S, HID), np.float32)
    for b in range(DP):
        acc = x[b].astype(np.float32).copy()
        for i in range(TP):
            acc += res.results[b * TP + i]["out"].astype(np.float32) * OSC
        out[b] = acc
    return out
